# revision 1
# baseline (speedup 1.0000x reference)
"""Trainium2 Bass kernel for nn_DilatedContextAttentionModule (B=8, C=256, 64x64).

Reference, per batch element (N = 64*64 = 4096):
    g   = G xj + g_b 1^T;  th = T xi + t_b 1^T;  phi = P xj + p_b 1^T
    f   = th^T phi / N                      (N x N, linear -- NO softmax)
    y[c,n] = sum_m f[n,m] g[c,m]
    z   = W y + W_b 1^T + xi
    out = BatchNorm2d(z)                    (training-mode batch stats)

Algebraic collapse (associativity; exact because f is linear):
    y = (1/N) (g phi^T) th = (1/N) S th,      S: C x C
    z = (E' + I) xi + d 1^T
    E' = (1/N) W S T,   d = (1/N) W S t_b + W_b
    S  = g0 phi0^T + (G sxj + N g_b) p_b^T + g_b (P sxj)^T
         (g0 = G xj, phi0 = P xj, sxj = xj @ 1)
This cuts ~9.7 GMAC/batch to ~0.9 GMAC/batch (the headroom of the problem).

Mapping to the NeuronCore (one batch element per core, 8 cores):
  phase 1  conv + S:  per 128-column chunk of n, one PSUM group computes
           [g0^T | phi0^T] (lhsT = xj chunk, n lands on partitions -- no
           transposes anywhere in the kernel), ACT copies PSUM->SBUF as
           float32r, then two matmuls accumulate S in PSUM across all 32
           chunks; the two bias rank-1 terms are K=1 matmuls.
  phase 2  E'^T = T^T (S^T W^T/N) + I and d via small matmuls; identity
           added by DVE during the PSUM->SBUF move.
  phase 3  z tiles [128, 512] = E_aug^T.T @ xi (+ d x ones row, K=1);
           ACT copies PSUM->SBUF; DVE bn_stats per tile (mean/var).
  BN       per-channel (mean, mean-of-squares)/8 packed [128, 2] per
           channel-chunk; ONE AllReduce per chunk -- chunk 0's collective
           + normalize + store overlap chunk 1's compute, so only the
           second collective's ~10 us floor lands on the critical path.
  stores   normalize in-place (DVE tensor_scalar) and DMA out per half.

TensorE dtype: float32r (fp32 bits streamed at 1 cycle/row for moving
free dim >= 256, vs 4 cycles/row for plain fp32; ~13-14 effective
mantissa bits). All matmul operand tiles are allocated float32r; the
producers (casting gpsimd DMAs, ACT/DVE copies) emit rounded values as
the walrus verifier requires. Measured end-to-end rms relative error vs
the fp32 jax reference: 2.3e-4 (plain fp32 build: 8.6e-7, ~2x slower).

Cost-model timeline: 71.5 us/core (collective priced as a local copy);
realistic HW estimate ~80 us/core including one unhidden 8-core
AllReduce floor (~9.7 us).
"""

import numpy as np

import concourse.bass as bass
import concourse.bacc as bacc
import concourse.tile as tile
from concourse import mybir
from concourse import bass_utils

B = 8
C = 256
N = 4096          # 64 * 64
NCORES = 8
NCH = 2           # channel chunks of 128
NT = 32           # n chunks of 128 (phase 1)
NZ = 8            # n tiles of 512 (phase 3)
F32 = mybir.dt.float32
BN_EPS = 1e-5

# TensorE compute dtype for the big matmuls. float32r streams at
# 1 cycle/row (vs 4 for float32) when the moving free dim >= 256, but
# requires all producers to round their outputs to float32r.
import os as _os
MM_DT = {
    "f32": mybir.dt.float32,
    "f32r": mybir.dt.float32r,
    "bf16": mybir.dt.bfloat16,
}[_os.environ.get("DCAM_MM_DT", "f32r")]


def _mm(x: bass.AP) -> bass.AP:
    # Tiles feeding matmuls are allocated as MM_DT directly; no-op now.
    return x


def build_kernel(nc, skip_cc: bool = False) -> None:
    f32 = F32
    xi_d = nc.dram_tensor("xi", [C, N], f32, kind="ExternalInput").ap()
    xj_d = nc.dram_tensor("xj", [C, N], f32, kind="ExternalInput").ap()
    # [128, 2, 512]: packed per-chunk conv weights [G^T | P^T]
    wgp_d = nc.dram_tensor("wgp", [128, NCH, 512], f32, kind="ExternalInput").ap()
    # [128, 2, 256]: theta_w rows (lhsT for E'^T), chunked on cp
    wtw_d = nc.dram_tensor("wtw", [128, NCH, C], f32, kind="ExternalInput").ap()
    # [128, 2, 256]: (W_w^T / N) rows, chunked on cg
    wwt_d = nc.dram_tensor("wwt", [128, NCH, C], f32, kind="ExternalInput").ap()
    # [128, 2]: theta_b column, chunked
    wtb_d = nc.dram_tensor("wtb", [128, NCH], f32, kind="ExternalInput").ap()
    # [1, 1024]: rows [N*g_b | g_b | p_b | W_b]
    aux_d = nc.dram_tensor("aux", [1, 4 * C + 512], f32, kind="ExternalInput").ap()
    # [128, 2, 2]: (gamma, beta) per channel, chunked
    gbe_d = nc.dram_tensor("gbe", [128, NCH, 2], f32, kind="ExternalInput").ap()
    # [128, 2]: W_b column, chunked
    wbc_d = nc.dram_tensor("wbc", [128, NCH], f32, kind="ExternalInput").ap()
    # [128, 2, 256]: identity matrix chunks (for z = (E'+I) xi + d 1^T)
    idn_d = nc.dram_tensor("idn", [128, NCH, C], f32, kind="ExternalInput").ap()
    out_d = nc.dram_tensor("out", [C, N], f32, kind="ExternalOutput").ap()

    with tile.TileContext(nc) as tc:
        _body(tc, xi_d, xj_d, wgp_d, wtw_d, wwt_d, wtb_d, aux_d, gbe_d, idn_d,
              wbc_d, out_d, skip_cc=skip_cc)


def _body(tc, xi_d, xj_d, wgp_d, wtw_d, wwt_d, wtb_d, aux_d, gbe_d, idn_d,
          wbc_d, out_d, skip_cc: bool = False):
    nc = tc.nc
    f32 = F32
    import contextlib

    with contextlib.ExitStack() as ctx:
        constp = ctx.enter_context(tc.tile_pool(name="const", bufs=1))
        datap = ctx.enter_context(tc.tile_pool(name="data", bufs=1))
        workp = ctx.enter_context(tc.tile_pool(name="work", bufs=4))
        rowsp = ctx.enter_context(tc.tile_pool(name="rows", bufs=2))
        psbig = ctx.enter_context(tc.tile_pool(name="ps_big", bufs=3, space="PSUM"))
        psacc = ctx.enter_context(tc.tile_pool(name="ps_acc", bufs=2, space="PSUM"))
        pssml = ctx.enter_context(tc.tile_pool(name="ps_sml", bufs=1, space="PSUM"))
        dramp = ctx.enter_context(tc.tile_pool(name="dram", bufs=2, space="DRAM"))

        # ---- constants / weights ------------------------------------
        mdt = MM_DT
        NQ = 4
        HN = N // NQ
        w_gp = constp.tile([128, NCH, 512], mdt, tag="w_gp")
        nc.gpsimd.dma_start(out=w_gp, in_=wgp_d)
        xj_h = []
        for h in range(NQ):
            t = datap.tile([128, NCH, HN], mdt, tag=f"xjh{h}", name=f"xj_h{h}")
            nc.gpsimd.dma_start(
                out=t,
                in_=xj_d.rearrange("(k p) n -> p k n", p=128)[:, :, h * HN:(h + 1) * HN],
            )
            xj_h.append(t)
        w_tw = constp.tile([128, NCH, C], mdt, tag="w_tw")
        nc.gpsimd.dma_start(out=w_tw, in_=wtw_d)
        w_wt = constp.tile([128, NCH, C], mdt, tag="w_wt")
        nc.gpsimd.dma_start(out=w_wt, in_=wwt_d)
        w_tb = constp.tile([128, NCH], mdt, tag="w_tb")
        nc.gpsimd.dma_start(out=w_tb, in_=wtb_d)
        aux = constp.tile([1, 4 * C + 512], mdt, tag="aux")
        nc.gpsimd.dma_start(out=aux, in_=aux_d)
        gbe = constp.tile([128, NCH, 2], f32, tag="gbe")
        nc.sync.dma_start(out=gbe, in_=gbe_d)
        wbc = constp.tile([128, NCH], f32, tag="wbc")
        nc.sync.dma_start(out=wbc, in_=wbc_d)
        idn = constp.tile([128, NCH, C], mdt, tag="idn")
        nc.gpsimd.dma_start(out=idn, in_=idn_d)
        eps = constp.tile([128, 1], f32, tag="eps")
        nc.vector.memset(eps, BN_EPS)

        # ---- big data tiles -----------------------------------------
        XHN = N // 2
        xi_h = []
        for h in range(2):
            t = datap.tile([128, NCH, XHN], mdt, tag=f"xih{h}", name=f"xi_h{h}")
            nc.gpsimd.dma_start(
                out=t,
                in_=xi_d.rearrange("(k p) n -> p k n", p=128)[:, :, h * XHN:(h + 1) * XHN],
            )
            xi_h.append(t)

        def xi_sl(k, tix):
            # phase-3 tile tix of 512 columns, channel-chunk k
            h, off = divmod(tix * 512, XHN)
            return xi_h[h][:, k, off:off + 512]

        def xj_sl(k, i):
            # phase-1 chunk i of 128 columns, channel-chunk k
            h, off = divmod(i * 128, HN)
            return xj_h[h][:, k, off:off + 128]

        # ---- sxj = rowsum(xj); bias-correction rows ------------------
        sxj = rowsp.tile([128, NCH], mdt, tag="sxj")
        sxjp = rowsp.tile([128, NCH, NQ], f32, tag="sxjp")
        with nc.allow_low_precision(reason="f32r output carries full fp32 bits"):
            for k in range(NCH):
                for h in range(NQ):
                    nc.vector.reduce_sum(
                        out=sxjp[:, k, h:h + 1], in_=xj_h[h][:, k, :],
                        axis=mybir.AxisListType.X,
                    )
                nc.vector.reduce_sum(
                    out=sxj[:, k:k + 1], in_=sxjp[:, k, :],
                    axis=mybir.AxisListType.X,
                )
        # s_g0_row = sxj^T @ G^T, s_phi0_row = sxj^T @ P^T   (each [1, 256])
        srow_ps = pssml.tile([1, 2 * C], f32, tag="sml")
        for k in range(NCH):
            nc.tensor.matmul(
                srow_ps[:, 0:C],
                _mm(sxj[:, k:k + 1]),
                _mm(w_gp[:, k, 0:C]),
                start=(k == 0), stop=(k == NCH - 1),
            )
        for k in range(NCH):
            nc.tensor.matmul(
                srow_ps[:, C:2 * C],
                _mm(sxj[:, k:k + 1]),
                _mm(w_gp[:, k, C:2 * C]),
                start=(k == 0), stop=(k == NCH - 1),
            )
        # u_row = s_g0 + N*g_b ; v_row = s_phi0
        urow = rowsp.tile([1, C], mdt, tag="urow")
        nc.vector.tensor_add(urow, srow_ps[:, 0:C], aux[:, 0:C])
        vrow = rowsp.tile([1, C], mdt, tag="vrow")
        nc.vector.tensor_copy(vrow, srow_ps[:, C:2 * C])

        # ---- phase 1: S = g0 phi0^T (+ rank-1 bias corrections) -----
        S_ps = [psacc.tile([128, C], f32, tag="acc", name=f"S_ps{m}") for m in range(NCH)]
        for i in range(NT):
            gp_ps = psbig.tile([128, 512], f32, tag="big")
            for k in range(NCH):
                nc.tensor.matmul(
                    gp_ps, _mm(xj_sl(k, i)), _mm(w_gp[:, k, :]),
                    start=(k == 0), stop=(k == NCH - 1),
                )
            gpt = workp.tile([128, 512], mdt, tag="gpt")
            if i >= 24:
                # late chunks: sxj is done, DVE has slack; shorten ACT chain
                nc.vector.tensor_copy(gpt, gp_ps)
            else:
                nc.scalar.copy(gpt, gp_ps)
            for m in range(NCH):
                nc.tensor.matmul(
                    S_ps[m],
                    _mm(gpt[:, m * 128:(m + 1) * 128]),
                    _mm(gpt[:, C:2 * C]),
                    start=(i == 0), stop=False,
                )
        for m in range(NCH):
            msl = slice(m * 128, (m + 1) * 128)
            nc.tensor.matmul(
                S_ps[m], _mm(urow[:, msl]), _mm(aux[:, 2 * C:3 * C]),
                start=False, stop=False,
            )
            nc.tensor.matmul(
                S_ps[m], _mm(aux[:, C + m * 128:C + (m + 1) * 128]), _mm(vrow),
                start=False, stop=True,
            )
        S_sb = []
        for m in range(NCH):
            t = workp.tile([128, C], mdt, tag=f"S{m}")
            nc.vector.tensor_copy(t, S_ps[m])
            S_sb.append(t)

        # ---- phase 2: V = S^T (W^T/N);  E'^T = T^T V;  d = V^T t_b --
        V_sb = []
        for m in range(NCH):
            v_ps = psacc.tile([128, C], f32, tag="acc")
            msl = slice(m * 128, (m + 1) * 128)
            for k in range(NCH):
                nc.tensor.matmul(
                    v_ps, _mm(S_sb[k][:, msl]), _mm(w_wt[:, k, :]),
                    start=(k == 0), stop=(k == NCH - 1),
                )
            t = workp.tile([128, C], mdt, tag=f"V{m}")
            nc.vector.tensor_copy(t, v_ps)
            V_sb.append(t)
        ET_sb = []
        for m in range(NCH):
            e_ps = psacc.tile([128, C], f32, tag="acc")
            msl = slice(m * 128, (m + 1) * 128)
            for k in range(NCH):
                nc.tensor.matmul(
                    e_ps, _mm(w_tw[:, k, msl]), _mm(V_sb[k]),
                    start=(k == 0), stop=(k == NCH - 1),
                )
            t = workp.tile([128, C], mdt, tag=f"ET{m}")
            nc.vector.tensor_add(t, e_ps, idn[:, m, :])
            ET_sb.append(t)
        dcol_ps = pssml.tile([128, NCH], f32, tag="sml")
        for j in range(NCH):
            for k in range(NCH):
                # N=1 moving dim: f32r is not ISA-legal here, use plain f32
                nc.tensor.matmul(
                    dcol_ps[:, j:j + 1],
                    V_sb[k][:, j * 128:(j + 1) * 128].bitcast(F32),
                    w_tb[:, k:k + 1].bitcast(F32),
                    start=(k == 0), stop=(k == NCH - 1),
                )
        dcol = rowsp.tile([128, NCH], f32, tag="dcol")
        nc.vector.tensor_add(dcol, dcol_ps, wbc)

        # ---- phase 3: z = (E'+I)^T.T @ xi + d 1^T; BN stats fused ---
        z_t = datap.tile([128, NCH, N], f32, tag="z")
        spack = rowsp.tile([128, 4], f32, tag="spack")
        ssum = rowsp.tile([128, 4], f32, tag="ssum")
        for j in range(NCH):
            jsl = slice(j * 128, (j + 1) * 128)
            stats = workp.tile([128, NZ, 6], f32, tag="bnst", name=f"stats{j}")
            for tix in range(NZ):
                tsl = slice(tix * 512, (tix + 1) * 512)
                z_ps = psbig.tile([128, 512], f32, tag="big")
                for k in range(NCH):
                    nc.tensor.matmul(
                        z_ps, _mm(ET_sb[k][:, jsl]), _mm(xi_sl(k, tix)),
                        start=(k == 0), stop=(k == NCH - 1),
                    )
                nc.scalar.activation(
                    out=z_t[:, j, tsl], in_=z_ps,
                    func=mybir.ActivationFunctionType.Identity,
                    bias=dcol[:, j:j + 1], scale=1.0,
                )
                nc.vector.bn_stats(out=stats[:, tix, :], in_=z_t[:, j, tsl])
            mv = rowsp.tile([128, 2], f32, tag="mv")
            nc.vector.bn_aggr(out=mv, in_=stats)
            nc.vector.tensor_scalar_mul(
                spack[:, 2 * j:2 * j + 1], mv[:, 0:1], 1.0 / NCORES)
            # (mean^2 + var) / NCORES  (= mean of squares, pre-scaled)
            nc.vector.scalar_tensor_tensor(
                out=spack[:, 2 * j + 1:2 * j + 2], in0=mv[:, 0:1],
                scalar=mv[:, 0:1], in1=mv[:, 1:2],
                op0=mybir.AluOpType.mult, op1=mybir.AluOpType.add,
            )
            nc.vector.tensor_scalar_mul(
                spack[:, 2 * j + 1:2 * j + 2],
                spack[:, 2 * j + 1:2 * j + 2], 1.0 / NCORES)
            cc_in = dramp.tile([128, 2], f32, tag=f"cc_in{j}", name=f"cc_in{j}")
            cc_out = dramp.tile([128, 2], f32, tag=f"cc_out{j}", name=f"cc_out{j}")
            nc.sync.dma_start(out=cc_in, in_=spack[:, 2 * j:2 * j + 2])
            if skip_cc:
                nc.sync.dma_start(out=cc_out, in_=cc_in)
            else:
                nc.gpsimd.collective_compute(
                    "AllReduce",
                    mybir.AluOpType.add,
                    replica_groups=[list(range(NCORES))],
                    ins=[cc_in.opt()],
                    outs=[cc_out.opt()],
                )
            nc.sync.dma_start(out=ssum[:, 2 * j:2 * j + 2], in_=cc_out)

            # ---- normalize + affine + store (inside j loop: chunk 0's
            # collective + store overlap chunk 1's compute) ------------
            mcol = ssum[:, 2 * j:2 * j + 1]
            qcol = ssum[:, 2 * j + 1:2 * j + 2]
            # negvar = m^2 - q  (sqrt uses scale=-1 to flip the sign)
            nvcol = rowsp.tile([128, 1], f32, tag="nvcol")
            nc.vector.scalar_tensor_tensor(
                out=nvcol, in0=mcol, scalar=mcol, in1=qcol,
                op0=mybir.AluOpType.mult, op1=mybir.AluOpType.subtract,
            )
            # rstd = 1 / sqrt(-negvar + eps) = 1 / sqrt(var + eps)
            scol = rowsp.tile([128, 1], f32, tag="scol")
            nc.scalar.activation(
                out=scol, in_=nvcol, func=mybir.ActivationFunctionType.Sqrt,
                bias=eps, scale=-1.0,
            )
            nc.vector.reciprocal(out=scol, in_=scol)
            acol = rowsp.tile([128, 1], f32, tag="acol")
            nc.vector.tensor_mul(acol, scol, gbe[:, j, 0:1])
            # nbcol = m*a - beta;  apply computes z*a - nbcol = z*a + beta - m*a
            bcol = rowsp.tile([128, 1], f32, tag="bcol")
            nc.vector.scalar_tensor_tensor(
                out=bcol, in0=mcol, scalar=acol, in1=gbe[:, j, 1:2],
                op0=mybir.AluOpType.mult, op1=mybir.AluOpType.subtract,
            )
            # apply z*a - nb in halves, each half split DVE || ACT so the
            # post-collective tail is half as long
            nbcol = rowsp.tile([128, 1], f32, tag="nbcol")
            nc.vector.tensor_scalar_mul(nbcol, bcol, -1.0)
            for h in range(2):
                hsl = slice(h * (N // 2), (h + 1) * (N // 2))
                q0 = slice(h * (N // 2), h * (N // 2) + N // 4)
                q1 = slice(h * (N // 2) + N // 4, (h + 1) * (N // 2))
                nc.vector.tensor_scalar(
                    out=z_t[:, j, q0], in0=z_t[:, j, q0],
                    scalar1=acol, scalar2=bcol,
                    op0=mybir.AluOpType.mult, op1=mybir.AluOpType.subtract,
                )
                nc.scalar.activation(
                    out=z_t[:, j, q1], in_=z_t[:, j, q1],
                    func=mybir.ActivationFunctionType.Identity,
                    bias=nbcol, scale=acol,
                )
                nc.sync.dma_start(
                    out=out_d[j * 128:(j + 1) * 128, hsl], in_=z_t[:, j, hsl])


_NC_CACHE: dict = {}


def _get_nc():
    if "nc" not in _NC_CACHE:
        nc = bacc.Bacc(
            "TRN2",
            target_bir_lowering=False,
            debug=False,
            enable_asserts=True,
            num_devices=NCORES,
        )
        build_kernel(nc)
        nc.compile()
        _NC_CACHE["nc"] = nc
    return _NC_CACHE["nc"]


def _make_in_maps(inputs: dict) -> list[dict]:
    xi = np.ascontiguousarray(np.asarray(inputs["xi"], np.float32).reshape(B, C, N))
    xj = np.ascontiguousarray(np.asarray(inputs["xj"], np.float32).reshape(B, C, N))
    g_w = np.asarray(inputs["g_w"], np.float32)
    g_b = np.asarray(inputs["g_b"], np.float32)
    t_w = np.asarray(inputs["theta_w"], np.float32)
    t_b = np.asarray(inputs["theta_b"], np.float32)
    p_w = np.asarray(inputs["phi_w"], np.float32)
    p_b = np.asarray(inputs["phi_b"], np.float32)
    W_w = np.asarray(inputs["W_w"], np.float32)
    W_b = np.asarray(inputs["W_b"], np.float32)
    gam = np.asarray(inputs["bn_gamma"], np.float32)
    bet = np.asarray(inputs["bn_beta"], np.float32)

    def chunked(a):  # [256, F] -> [128, 2, F]
        return np.ascontiguousarray(a.reshape(2, 128, -1).transpose(1, 0, 2))

    wgp = chunked(np.concatenate([g_w.T, p_w.T], axis=1))          # [128,2,512]
    wtw = chunked(t_w)                                             # [128,2,256]
    wwt = chunked(W_w.T * (1.0 / N))                               # [128,2,256]
    wtb = np.ascontiguousarray(t_b.reshape(2, 128).T)              # [128,2]
    aux = np.concatenate([N * g_b, g_b, p_b, W_b,
                          np.ones(512, np.float32)])[None, :]   # [1,1536]
    aux = np.ascontiguousarray(aux.astype(np.float32))
    gbe = chunked(np.stack([gam, bet], axis=1))                    # [128,2,2]
    idn = chunked(np.eye(C, dtype=np.float32))                     # [128,2,256]
    wbc = np.ascontiguousarray(W_b.reshape(2, 128).T)              # [128,2]

    in_maps = []
    for b in range(B):
        in_maps.append({
            "xi": xi[b], "xj": xj[b],
            "wgp": wgp, "wtw": wtw, "wwt": wwt, "wtb": wtb,
            "aux": aux, "gbe": gbe, "idn": idn, "wbc": wbc,
        })
    return in_maps


def kernel(**inputs) -> np.ndarray:
    nc = _get_nc()
    in_maps = _make_in_maps(inputs)
    last_err = None
    for attempt in range(3):
        try:
            res = bass_utils.run_bass_kernel_spmd(
                nc, in_maps, core_ids=list(range(NCORES)),
            )
            break
        except Exception as e:  # transient device wedge: back off and retry
            last_err = e
            import time as _time
            _time.sleep(4.0 * (attempt + 1))
            try:
                import jax
                import jax.extend.backend as _jeb
                jax.clear_caches()
                # tear down the PJRT client: a fresh axon connection lets the
                # terminal reset a wedged exec unit
                _jeb.clear_backends()
            except Exception:
                pass
    else:
        raise last_err
    out = np.stack([res.results[c]["out"] for c in range(NCORES)])
    return np.ascontiguousarray(out.reshape(B, C, 64, 64).astype(np.float32))


if __name__ == "__main__":
    rng = np.random.default_rng(0)
    fake = {
        "xi": rng.standard_normal((B, C, 64, 64), np.float32),
        "xj": rng.standard_normal((B, C, 64, 64), np.float32),
        "g_w": rng.standard_normal((C, C), np.float32) / 16,
        "g_b": rng.standard_normal((C,), np.float32) / 16,
        "theta_w": rng.standard_normal((C, C), np.float32) / 16,
        "theta_b": rng.standard_normal((C,), np.float32) / 16,
        "phi_w": rng.standard_normal((C, C), np.float32) / 16,
        "phi_b": rng.standard_normal((C,), np.float32) / 16,
        "W_w": rng.standard_normal((C, C), np.float32) / 16,
        "W_b": rng.standard_normal((C,), np.float32) / 16,
        "bn_gamma": np.ones((C,), np.float32),
        "bn_beta": np.zeros((C,), np.float32),
    }
    out = kernel(**fake)
    print("out", out.shape, out.dtype, float(np.abs(out).mean()))



# revision 6
# speedup vs baseline: 1.6889x; 1.6889x over previous
"""Trainium2 Bass kernel for nn_DilatedContextAttentionModule (B=8, C=256, 64x64).

Reference, per batch element (N = 64*64 = 4096):
    g   = G xj + g_b 1^T;  th = T xi + t_b 1^T;  phi = P xj + p_b 1^T
    f   = th^T phi / N                      (N x N, linear -- NO softmax)
    y[c,n] = sum_m f[n,m] g[c,m]
    z   = W y + W_b 1^T + xi
    out = BatchNorm2d(z)                    (training-mode batch stats)

Algebraic collapse v3 (Gram-matrix form; exact because f is linear):
    z = (I + E') xi + d 1^T
    E' = L K R + a1 b1^T + a2 b2^T,  K = xj xj^T  (C x C Gram)
    with host-folded constants
      L' = W G (device uses K/N),  R = P^T T,  wgb = W g_b,
      b1 = T^T p_b,  ptb = P^T t_b,  c1 = p_b . t_b
    and runtime vectors from sxj = xj @ 1:
      a1 = L'sxj/N + wgb,  b2 = R^T sxj  (a2 = wgb/N folded into b2/N)
      d  = L'(K/N)ptb + c1 a1 + (sxj.ptb/N) wgb + W_b
    ~0.55 GMAC/batch vs 9.7 GMAC for the naive attention (headroom=9).

Mapping to the NeuronCore (one batch element per core, 8 cores):
  - xj arrives HOST-TRANSPOSED (layout-only) as f16 so K = xj xj^T is 64
    plain matmuls with n on partitions; xi and the output are f16 too:
    the cost model's DMA path is one serial ~275 GB/s device, so bytes
    are the dominant resource. End-to-end rms vs fp32 jax: ~4.7e-4.
  - z pass 1: matmul z0 = A xi into PSUM; DVE bn_stats directly on PSUM
    (no copy-out; the +d bias shifts the mean analytically: var is
    shift-invariant). Pack (mean/8, meansq/8) for both chunks [128,4].
  - BN cross-core reduction: ONE ReduceScatter (cost-model floor 15 us
    vs 28 us AllReduce): input = own stats tiled 8x (stride-0 DMA), so
    every core's scattered block is the full global sum.
  - z pass 2 (runs during the collective): recompute the z matmuls --
    TensorE is idle anyway -- and fuse the whole BN affine into the
    PSUM->SBUF copy (out = a*z0 + e, e = a*(d-mean)+beta), alternating
    DVE/ACT per tile, storing each f16 tile as soon as it is ready.
"""

import numpy as np

import concourse.bass as bass
import concourse.bacc as bacc
import concourse.tile as tile
from concourse import mybir
from concourse import bass_utils

B = 8
C = 256
N = 4096          # 64 * 64
NCORES = 8
NCH = 2           # channel chunks of 128
NT = 32           # n chunks of 128 (Gram phase)
NZ = 8            # n tiles of 512 (z phase)
F32 = mybir.dt.float32
F16 = mybir.dt.float16
BN_EPS = 1e-5

# wmat layout (f16, [128, 2, 770]): per channel-chunk k:
#   [0:256]   L'^T rows   (lt)
#   [256:512] R rows      (rc)
#   [512:768] identity    (idn)
#   [768:770] ptb column (only col 768 used: ptb[k*128+p])
WM_LT = slice(0, 256)
WM_RC = slice(256, 512)
WM_ID = slice(512, 768)
WM_F = 770


def build_kernel(nc, skip_cc: bool = False) -> None:
    f32, f16 = F32, F16
    xjt_d = nc.dram_tensor("xjt", [128, NT, C], f16, kind="ExternalInput").ap()
    xi_d = nc.dram_tensor("xi", [128, NCH, N], f16, kind="ExternalInput").ap()
    wm_d = nc.dram_tensor("wm", [128, NCH, WM_F], f16, kind="ExternalInput").ap()
    # aux row: [b1 (256) | wgb (256) | c1 (1) | pad]
    aux_d = nc.dram_tensor("aux", [1, 2 * C + 8], f16, kind="ExternalInput").ap()
    # f32 smalls: [gamma | beta | W_b] columns  -> [128, 2, 3]
    sm_d = nc.dram_tensor("sm", [128, NCH, 3], f32, kind="ExternalInput").ap()
    out_d = nc.dram_tensor("out", [C, N], f16, kind="ExternalOutput").ap()

    with tile.TileContext(nc) as tc:
        _body(tc, xjt_d, xi_d, wm_d, aux_d, sm_d, out_d, skip_cc=skip_cc)


def _body(tc, xjt_d, xi_d, wm_d, aux_d, sm_d, out_d, skip_cc: bool = False):
    nc = tc.nc
    f32, f16 = F32, F16
    import contextlib

    with contextlib.ExitStack() as ctx:
        constp = ctx.enter_context(tc.tile_pool(name="const", bufs=1))
        datap = ctx.enter_context(tc.tile_pool(name="data", bufs=1))
        workp = ctx.enter_context(tc.tile_pool(name="work", bufs=4))
        rowsp = ctx.enter_context(tc.tile_pool(name="rows", bufs=2))
        outp = ctx.enter_context(tc.tile_pool(name="out", bufs=4))
        psbig = ctx.enter_context(tc.tile_pool(name="ps_big", bufs=3, space="PSUM"))
        psacc = ctx.enter_context(tc.tile_pool(name="ps_acc", bufs=2, space="PSUM"))
        psrow = ctx.enter_context(tc.tile_pool(name="ps_row", bufs=1, space="PSUM"))
        pscol = ctx.enter_context(tc.tile_pool(name="ps_col", bufs=1, space="PSUM"))
        dramp = ctx.enter_context(tc.tile_pool(name="dram", bufs=2, space="DRAM"))

        # ---- loads, in DMA-priority order (DMA is one serial device) ----
        xjt = datap.tile([128, NT, C], f16, tag="xjt")
        NXJ = 4
        for h in range(NXJ):
            sl = slice(h * (NT // NXJ), (h + 1) * (NT // NXJ))
            nc.sync.dma_start(out=xjt[:, sl, :], in_=xjt_d[:, sl, :])
        wm = constp.tile([128, NCH, WM_F], f16, tag="wm")
        nc.sync.dma_start(out=wm, in_=wm_d)
        aux = constp.tile([1, 2 * C + 8], f16, tag="aux")
        nc.sync.dma_start(out=aux, in_=aux_d)
        xi_t = datap.tile([128, NCH, N], f16, tag="xi")
        NXI = 4
        for h in range(NXI):
            sl = slice(h * (N // NXI), (h + 1) * (N // NXI))
            nc.sync.dma_start(out=xi_t[:, :, sl], in_=xi_d[:, :, sl])
        sm = constp.tile([128, NCH, 3], f32, tag="sm")
        nc.sync.dma_start(out=sm, in_=sm_d)

        ones = constp.tile([128, 1], f16, tag="ones")
        nc.vector.memset(ones, 1.0)
        eps = constp.tile([128, 1], f32, tag="eps")
        nc.vector.memset(eps, BN_EPS)

        def lt(k, csl=slice(0, C)):
            return wm[:, k, WM_LT][:, csl]

        def rcw(k, csl=slice(0, C)):
            return wm[:, k, WM_RC][:, csl]

        def ptbc(k):
            return wm[:, k, 768:769]

        # ---- phase A: K = xj xj^T and sxj = xj @ 1 --------------------
        K_ps = [psacc.tile([128, C + 8], f32, tag="acc", name=f"K_ps{m}")
                for m in range(NCH)]
        for i in range(NT):
            for m in range(NCH):
                msl = slice(m * 128, (m + 1) * 128)
                nc.tensor.matmul(
                    K_ps[m][:, 0:C], xjt[:, i, msl], xjt[:, i, :],
                    start=(i == 0), stop=(i == NT - 1),
                )
        # sxj accumulates in col C of the same tiles as a second,
        # sequential PSUM group (concurrent groups per region are illegal)
        for i in range(NT):
            for m in range(NCH):
                msl = slice(m * 128, (m + 1) * 128)
                nc.tensor.matmul(
                    K_ps[m][:, C:C + 1], xjt[:, i, msl], ones,
                    start=(i == 0), stop=(i == NT - 1),
                )
        K_sb = []
        sxjc = rowsp.tile([128, NCH], f16, tag="sxjc")
        for m in range(NCH):
            t = workp.tile([128, C], f16, tag=f"K{m}")
            nc.scalar.activation(
                out=t, in_=K_ps[m][:, 0:C],
                func=mybir.ActivationFunctionType.Identity, scale=1.0 / N)
            K_sb.append(t)
            nc.vector.tensor_copy(sxjc[:, m:m + 1], K_ps[m][:, C:C + 1])

        # ---- phase B: runtime rows a1, b2, scalar c2 ------------------
        rows_ps = psrow.tile([1, 2 * C + 8], f32, tag="rows")
        for k in range(NCH):
            nc.tensor.matmul(rows_ps[:, 0:C], sxjc[:, k:k + 1], lt(k),
                             start=(k == 0), stop=(k == NCH - 1))
        for k in range(NCH):
            nc.tensor.matmul(rows_ps[:, C:2 * C], sxjc[:, k:k + 1], rcw(k),
                             start=(k == 0), stop=(k == NCH - 1))
        for k in range(NCH):
            nc.tensor.matmul(rows_ps[:, 2 * C:2 * C + 1], sxjc[:, k:k + 1],
                             ptbc(k),
                             start=(k == 0), stop=(k == NCH - 1))
        a1row = rowsp.tile([1, C], f16, tag="a1row")
        nc.vector.scalar_tensor_tensor(
            out=a1row, in0=rows_ps[:, 0:C], scalar=1.0 / N,
            in1=aux[:, C:2 * C],
            op0=mybir.AluOpType.mult, op1=mybir.AluOpType.add)
        b2row = rowsp.tile([1, C], f16, tag="b2row")
        nc.vector.tensor_scalar_mul(b2row, rows_ps[:, C:2 * C], 1.0 / N)
        c2cell = rowsp.tile([1, 1], f16, tag="c2cell")
        nc.vector.tensor_scalar_mul(c2cell, rows_ps[:, 2 * C:2 * C + 1], 1.0 / N)

        # ---- phase C: T1 = (K/N) L'^T;  ET = R^T T1 + rank1 + I -------
        T1_sb = []
        for cb in range(NCH):
            t1_ps = psacc.tile([128, C], f32, tag="acc")
            csl = slice(cb * 128, (cb + 1) * 128)
            for jb in range(NCH):
                nc.tensor.matmul(t1_ps, K_sb[jb][:, csl], lt(jb),
                                 start=(jb == 0), stop=(jb == NCH - 1))
            t = workp.tile([128, C], f16, tag=f"T1{cb}")
            nc.scalar.copy(t, t1_ps)
            T1_sb.append(t)
        ET_sb = []
        for ob in range(NCH):
            et_ps = psacc.tile([128, C], f32, tag="acc")
            osl = slice(ob * 128, (ob + 1) * 128)
            for cb in range(NCH):
                nc.tensor.matmul(et_ps, rcw(cb, osl), T1_sb[cb],
                                 start=(cb == 0), stop=False)
            nc.tensor.matmul(et_ps, aux[:, osl], a1row, start=False, stop=False)
            nc.tensor.matmul(et_ps, b2row[:, osl], aux[:, C:2 * C],
                             start=False, stop=True)
            t = workp.tile([128, C], f16, tag=f"ET{ob}")
            nc.vector.tensor_add(t, et_ps, wm[:, ob, WM_ID])
            ET_sb.append(t)

        # ---- phase D: d = L'(K/N)ptb + c1 a1 + c2n wgb + W_b ----------
        col_ps = pscol.tile([128, 8], f32, tag="cols")
        for cb in range(NCH):
            csl = slice(cb * 128, (cb + 1) * 128)
            for jb in range(NCH):
                nc.tensor.matmul(col_ps[:, cb:cb + 1], K_sb[jb][:, csl],
                                 ptbc(jb),
                                 start=(jb == 0), stop=(jb == NCH - 1))
        kpc = rowsp.tile([128, NCH], f16, tag="kpc")
        nc.scalar.copy(kpc, col_ps[:, 0:NCH])
        for ob in range(NCH):
            osl = slice(ob * 128, (ob + 1) * 128)
            for cb in range(NCH):
                nc.tensor.matmul(col_ps[:, 2 + ob:3 + ob], lt(cb, osl),
                                 kpc[:, cb:cb + 1],
                                 start=(cb == 0), stop=False)
            nc.tensor.matmul(col_ps[:, 2 + ob:3 + ob], a1row[:, osl],
                             aux[:, 2 * C:2 * C + 1], start=False, stop=False)
            nc.tensor.matmul(col_ps[:, 2 + ob:3 + ob],
                             aux[:, C + ob * 128:C + (ob + 1) * 128],
                             c2cell, start=False, stop=True)
        dcol = rowsp.tile([128, NCH], f32, tag="dcol")
        nc.vector.tensor_add(dcol, col_ps[:, 2:2 + NCH], sm[:, :, 2])

        # ---- phase E-a: z0 = A xi in PSUM; bn_stats straight off PSUM -
        stats = [workp.tile([128, NZ, 6], f32, tag="bnst", name=f"stats{j}")
                 for j in range(NCH)]
        for tix in range(NZ):
            tsl = slice(tix * 512, (tix + 1) * 512)
            for j in range(NCH):
                jsl = slice(j * 128, (j + 1) * 128)
                z_ps = psbig.tile([128, 512], f32, tag="big")
                for k in range(NCH):
                    nc.tensor.matmul(z_ps, ET_sb[k][:, jsl], xi_t[:, k, tsl],
                                     start=(k == 0), stop=(k == NCH - 1))
                nc.vector.bn_stats(out=stats[j][:, tix, :], in_=z_ps)

        # stats of z = z0 + d: mean += d, var unchanged.
        # spack = (mean/8, (var + mean^2)/8) per chunk -> [128, 4]
        spack = rowsp.tile([128, 4], f32, tag="spack")
        mcols = rowsp.tile([128, NCH], f32, tag="mcols")
        for j in range(NCH):
            mv = rowsp.tile([128, 2], f32, tag="mv")
            nc.vector.bn_aggr(out=mv, in_=stats[j])
            nc.vector.tensor_add(mcols[:, j:j + 1], mv[:, 0:1], dcol[:, j:j + 1])
            nc.vector.tensor_scalar_mul(
                spack[:, 2 * j:2 * j + 1], mcols[:, j:j + 1], 1.0 / NCORES)
            nc.vector.scalar_tensor_tensor(
                out=spack[:, 2 * j + 1:2 * j + 2], in0=mcols[:, j:j + 1],
                scalar=mcols[:, j:j + 1], in1=mv[:, 1:2],
                op0=mybir.AluOpType.mult, op1=mybir.AluOpType.add)
            nc.vector.tensor_scalar_mul(
                spack[:, 2 * j + 1:2 * j + 2],
                spack[:, 2 * j + 1:2 * j + 2], 1.0 / NCORES)

        # ---- ONE ReduceScatter: input = own stats tiled 8x ------------
        cc_in = dramp.tile([NCORES * 128, 4], f32, tag="cc_in", name="cc_in")
        cc_out = dramp.tile([128, 4], f32, tag="cc_out", name="cc_out")
        nc.sync.dma_start(
            out=cc_in.rearrange("(r p) f -> p r f", p=128),
            in_=spack.unsqueeze(1).broadcast_to([128, NCORES, 4]))
        if skip_cc:
            nc.sync.dma_start(out=cc_out, in_=cc_in[0:128, :])
        else:
            nc.gpsimd.collective_compute(
                "ReduceScatter",
                mybir.AluOpType.add,
                replica_groups=[list(range(NCORES))],
                ins=[cc_in.opt()],
                outs=[cc_out.opt()],
            )
        ssum = rowsp.tile([128, 4], f32, tag="ssum")
        nc.sync.dma_start(out=ssum, in_=cc_out)

        # ---- per-chunk affine constants: a, e = a*(d-mean)+beta -------
        acols = rowsp.tile([128, NCH], f32, tag="acols")
        ecols = rowsp.tile([128, NCH], f32, tag="ecols")
        for j in range(NCH):
            mcol = ssum[:, 2 * j:2 * j + 1]
            qcol = ssum[:, 2 * j + 1:2 * j + 2]
            nvcol = rowsp.tile([128, 1], f32, tag="nvcol")
            nc.vector.scalar_tensor_tensor(
                out=nvcol, in0=mcol, scalar=mcol, in1=qcol,
                op0=mybir.AluOpType.mult, op1=mybir.AluOpType.subtract)
            scol = rowsp.tile([128, 1], f32, tag="scol")
            nc.scalar.activation(
                out=scol, in_=nvcol, func=mybir.ActivationFunctionType.Sqrt,
                bias=eps, scale=-1.0)
            nc.vector.reciprocal(out=scol, in_=scol)
            nc.vector.tensor_mul(acols[:, j:j + 1], scol, sm[:, j, 0:1])
            # e = a*(d - M) + beta
            tcol = rowsp.tile([128, 1], f32, tag="tcol")
            nc.vector.scalar_tensor_tensor(
                out=tcol, in0=dcol[:, j:j + 1], scalar=1.0, in1=mcol,
                op0=mybir.AluOpType.mult, op1=mybir.AluOpType.subtract)
            nc.vector.scalar_tensor_tensor(
                out=ecols[:, j:j + 1], in0=tcol, scalar=acols[:, j:j + 1],
                in1=sm[:, j, 1:2],
                op0=mybir.AluOpType.mult, op1=mybir.AluOpType.add)

        # ---- phase E-b: recompute z0, fuse BN affine, store f16 -------
        for tix in range(NZ):
            tsl = slice(tix * 512, (tix + 1) * 512)
            for j in range(NCH):
                jsl = slice(j * 128, (j + 1) * 128)
                z_ps = psbig.tile([128, 512], f32, tag="big")
                for k in range(NCH):
                    nc.tensor.matmul(z_ps, ET_sb[k][:, jsl], xi_t[:, k, tsl],
                                     start=(k == 0), stop=(k == NCH - 1))
                o16 = outp.tile([128, 512], f16, tag="o16")
                if (tix + j) % 2 == 0:
                    nc.vector.tensor_scalar(
                        out=o16, in0=z_ps,
                        scalar1=acols[:, j:j + 1], scalar2=ecols[:, j:j + 1],
                        op0=mybir.AluOpType.mult, op1=mybir.AluOpType.add)
                else:
                    nc.scalar.activation(
                        out=o16, in_=z_ps,
                        func=mybir.ActivationFunctionType.Identity,
                        bias=ecols[:, j:j + 1], scale=acols[:, j:j + 1])
                nc.sync.dma_start(
                    out=out_d[j * 128:(j + 1) * 128, tsl], in_=o16)


_NC_CACHE: dict = {}


def _get_nc():
    if "nc" not in _NC_CACHE:
        nc = bacc.Bacc(
            "TRN2",
            target_bir_lowering=False,
            debug=False,
            enable_asserts=True,
            num_devices=NCORES,
        )
        build_kernel(nc)
        nc.compile()
        _NC_CACHE["nc"] = nc
    return _NC_CACHE["nc"]


def _make_in_maps(inputs: dict) -> list[dict]:
    xi = np.asarray(inputs["xi"], np.float32).reshape(B, C, N)
    xj = np.asarray(inputs["xj"], np.float32).reshape(B, C, N)
    g_w = np.asarray(inputs["g_w"], np.float32)
    g_b = np.asarray(inputs["g_b"], np.float32)
    t_w = np.asarray(inputs["theta_w"], np.float32)
    t_b = np.asarray(inputs["theta_b"], np.float32)
    p_w = np.asarray(inputs["phi_w"], np.float32)
    p_b = np.asarray(inputs["phi_b"], np.float32)
    W_w = np.asarray(inputs["W_w"], np.float32)
    W_b = np.asarray(inputs["W_b"], np.float32)
    gam = np.asarray(inputs["bn_gamma"], np.float32)
    bet = np.asarray(inputs["bn_beta"], np.float32)

    def chunked(a):  # [256, F] -> [128, 2, F]
        return np.ascontiguousarray(a.reshape(2, 128, -1).transpose(1, 0, 2))

    # host-folded weight products (constant folding, fp32)
    Lp = W_w @ g_w                      # L' = W G   (device uses K/N)
    R = p_w.T @ t_w                     # R = P^T T
    wgb = W_w @ g_b
    b1 = t_w.T @ p_b
    ptb = p_w.T @ t_b
    c1 = float(p_b @ t_b)

    wm = np.zeros((128, NCH, WM_F), np.float16)
    wm[:, :, 0:C] = chunked(Lp.T)
    wm[:, :, C:2 * C] = chunked(R)
    wm[:, :, 2 * C:3 * C] = chunked(np.eye(C, dtype=np.float32))
    wm[:, :, 3 * C] = ptb.reshape(2, 128).T
    aux = np.zeros((1, 2 * C + 8), np.float16)
    aux[0, 0:C] = b1.astype(np.float16)
    aux[0, C:2 * C] = wgb.astype(np.float16)
    aux[0, 2 * C] = np.float16(c1)
    sm = np.zeros((128, NCH, 3), np.float32)
    sm[:, :, 0] = gam.reshape(2, 128).T
    sm[:, :, 1] = bet.reshape(2, 128).T
    sm[:, :, 2] = W_b.reshape(2, 128).T

    in_maps = []
    for b in range(B):
        # layout-only transforms of the per-batch data (f16)
        xjt = np.ascontiguousarray(
            xj[b].T.reshape(NT, 128, C).transpose(1, 0, 2)).astype(np.float16)
        xib = chunked(xi[b]).astype(np.float16)      # [128,2,4096]
        in_maps.append({
            "xjt": xjt, "xi": xib, "wm": wm, "aux": aux, "sm": sm,
        })
    return in_maps


def kernel(**inputs) -> np.ndarray:
    nc = _get_nc()
    in_maps = _make_in_maps(inputs)
    last_err = None
    for attempt in range(3):
        try:
            res = bass_utils.run_bass_kernel_spmd(
                nc, in_maps, core_ids=list(range(NCORES)),
            )
            break
        except Exception as e:  # transient device wedge: back off and retry
            last_err = e
            import time as _time
            _time.sleep(4.0 * (attempt + 1))
            try:
                import jax
                import jax.extend.backend as _jeb
                jax.clear_caches()
                _jeb.clear_backends()
            except Exception:
                pass
    else:
        raise last_err
    out = np.stack([res.results[c]["out"] for c in range(NCORES)])
    return np.ascontiguousarray(out.reshape(B, C, 64, 64).astype(np.float32))


if __name__ == "__main__":
    rng = np.random.default_rng(0)
    fake = {
        "xi": rng.standard_normal((B, C, 64, 64)).astype(np.float32),
        "xj": rng.standard_normal((B, C, 64, 64)).astype(np.float32),
        "g_w": (rng.standard_normal((C, C)) / 16).astype(np.float32),
        "g_b": (rng.standard_normal((C,)) / 16).astype(np.float32),
        "theta_w": (rng.standard_normal((C, C)) / 16).astype(np.float32),
        "theta_b": (rng.standard_normal((C,)) / 16).astype(np.float32),
        "phi_w": (rng.standard_normal((C, C)) / 16).astype(np.float32),
        "phi_b": (rng.standard_normal((C,)) / 16).astype(np.float32),
        "W_w": (rng.standard_normal((C, C)) / 16).astype(np.float32),
        "W_b": (rng.standard_normal((C,)) / 16).astype(np.float32),
        "bn_gamma": np.ones((C,), np.float32),
        "bn_beta": np.zeros((C,), np.float32),
    }
    out = kernel(**fake)
    print("out", out.shape, out.dtype, float(np.abs(out).mean()))


# revision 7
# speedup vs baseline: 1.9675x; 1.1649x over previous
"""Trainium2 Bass kernel for nn_DilatedContextAttentionModule (B=8, C=256, 64x64).

Reference, per batch element (N = 64*64 = 4096):
    g   = G xj + g_b 1^T;  th = T xi + t_b 1^T;  phi = P xj + p_b 1^T
    f   = th^T phi / N                      (N x N, linear -- NO softmax)
    y[c,n] = sum_m f[n,m] g[c,m]
    z   = W y + W_b 1^T + xi
    out = BatchNorm2d(z)                    (training-mode batch stats)

Algebraic collapse (Gram-matrix form; exact because f is linear):
    z = (I + E') xi + d 1^T
    E' = L K R + a1 b1^T + a2 b2^T,  K = xj xj^T  (C x C Gram)
    with host-folded constants
      L' = W G (device uses K/N),  R = P^T T,  wgb = W g_b,
      b1 = T^T p_b,  ptb = P^T t_b,  c1 = p_b . t_b
    and runtime vectors from sxj = xj @ 1:
      a1 = L'sxj/N + wgb,  b2 = R^T sxj  (a2 = wgb/N folded into b2/N)
      d  = L'(K/N)ptb + c1 a1 + (sxj.ptb/N) wgb + W_b
    ~0.55 GMAC/batch vs 9.7 GMAC for the naive attention (headroom=9).

Mapping to the NeuronCore (one batch element per core, 8 cores):
  - xj arrives HOST-TRANSPOSED (layout-only) as f16 with a ones column
    appended, so ONE set of Gram matmuls yields both K = xj xj^T and
    sxj = xj @ 1 (K_aug = [xj|1]^T[xj|1]).  xi and the output are f16:
    the cost model's DMA path is one serial ~275 GB/s device, so bytes
    are the dominant resource.  End-to-end rms vs fp32 jax: ~4.7e-4.
  - a short warm-up matmul burst holds the PE p-state at full clock so
    the DMA-paced Gram matmuls don't run at the 0.65 GHz cold clock.
  - z pass: matmul z0 = A xi into PSUM; ACT applies the +d bias while
    copying to SBUF; DVE bn_stats runs directly on the same PSUM tile
    in parallel (mean shift by d corrected analytically; var is
    shift-invariant).
  - BN cross-core reduction: ONE ReduceScatter (cost-model floor 15 us
    vs 28 us AllReduce): input = own stats tiled 8x (stride-0 DMA), so
    every core's scattered block is the full global sum.
  - normalize: out = a*z - b fused per quarter, alternating DVE/ACT,
    each quarter stored as f16 as soon as it is ready.
"""

import numpy as np

import concourse.bass as bass
import concourse.bacc as bacc
import concourse.tile as tile
from concourse import mybir
from concourse import bass_utils

B = 8
C = 256
N = 4096          # 64 * 64
NCORES = 8
NCH = 2           # channel chunks of 128
NT = 32           # n chunks of 128 (Gram phase)
NZ = 8            # n tiles of 512 (z phase)
XJF = 264         # xjt free width: 256 channels | ones | pad
F32 = mybir.dt.float32
F16 = mybir.dt.float16
BN_EPS = 1e-5

# wmat layout (f16, [128, 2, 770]): per channel-chunk k:
#   [0:256] L'^T rows | [256:512] R rows | [512:768] identity | [768] ptb
WM_LT = slice(0, 256)
WM_RC = slice(256, 512)
WM_ID = slice(512, 768)
WM_F = 770


def build_kernel(nc, skip_cc: bool = False) -> None:
    f32, f16 = F32, F16
    xjt_d = nc.dram_tensor("xjt", [128, NT, XJF], f16, kind="ExternalInput").ap()
    xi_d = nc.dram_tensor("xi", [128, NCH, N], f16, kind="ExternalInput").ap()
    wm_d = nc.dram_tensor("wm", [128, NCH, WM_F], f16, kind="ExternalInput").ap()
    # aux row: [b1 (256) | wgb (256) | c1 (1) | pad]
    aux_d = nc.dram_tensor("aux", [1, 2 * C + 8], f16, kind="ExternalInput").ap()
    # f32 smalls: [gamma | beta | W_b] columns  -> [128, 2, 3]
    sm_d = nc.dram_tensor("sm", [128, NCH, 3], f32, kind="ExternalInput").ap()
    out_d = nc.dram_tensor("out", [C, N], f16, kind="ExternalOutput").ap()

    with tile.TileContext(nc) as tc:
        _body(tc, xjt_d, xi_d, wm_d, aux_d, sm_d, out_d, skip_cc=skip_cc)


def _body(tc, xjt_d, xi_d, wm_d, aux_d, sm_d, out_d, skip_cc: bool = False):
    nc = tc.nc
    f32, f16 = F32, F16
    import contextlib

    with contextlib.ExitStack() as ctx:
        constp = ctx.enter_context(tc.tile_pool(name="const", bufs=1))
        datap = ctx.enter_context(tc.tile_pool(name="data", bufs=1))
        workp = ctx.enter_context(tc.tile_pool(name="work", bufs=4))
        rowsp = ctx.enter_context(tc.tile_pool(name="rows", bufs=2))
        outp = ctx.enter_context(tc.tile_pool(name="out", bufs=4))
        psbig = ctx.enter_context(tc.tile_pool(name="ps_big", bufs=3, space="PSUM"))
        psacc = ctx.enter_context(tc.tile_pool(name="ps_acc", bufs=2, space="PSUM"))
        psrow = ctx.enter_context(tc.tile_pool(name="ps_row", bufs=1, space="PSUM"))
        pscol = ctx.enter_context(tc.tile_pool(name="ps_col", bufs=1, space="PSUM"))
        dramp = ctx.enter_context(tc.tile_pool(name="dram", bufs=2, space="DRAM"))

        # ---- loads, in DMA-priority order (DMA is one serial device) ----
        xjt = datap.tile([128, NT, XJF], f16, tag="xjt")
        NXJ = 8
        for h in range(NXJ):
            sl = slice(h * (NT // NXJ), (h + 1) * (NT // NXJ))
            nc.sync.dma_start(out=xjt[:, sl, :], in_=xjt_d[:, sl, :])
        wm = constp.tile([128, NCH, WM_F], f16, tag="wm")
        nc.sync.dma_start(out=wm, in_=wm_d)
        aux = constp.tile([1, 2 * C + 8], f16, tag="aux")
        nc.sync.dma_start(out=aux, in_=aux_d)
        xi_t = datap.tile([128, NCH, N], f16, tag="xi")
        NXI = 4
        for h in range(NXI):
            sl = slice(h * (N // NXI), (h + 1) * (N // NXI))
            nc.sync.dma_start(out=xi_t[:, :, sl], in_=xi_d[:, :, sl])
        sm = constp.tile([128, NCH, 3], f32, tag="sm")
        nc.sync.dma_start(out=sm, in_=sm_d)

        eps = constp.tile([128, 1], f32, tag="eps")
        nc.vector.memset(eps, BN_EPS)

        def lt(k, csl=slice(0, C)):
            return wm[:, k, WM_LT][:, csl]

        def rcw(k, csl=slice(0, C)):
            return wm[:, k, WM_RC][:, csl]

        def ptbc(k):
            return wm[:, k, 768:769]

        # ---- PE warm-up: hold the p-state at full clock until the ----
        # ---- first Gram chunk arrives (cold PE runs at 0.65 GHz)  ----
        warm = constp.tile([128, 640], f16, tag="warm")
        nc.vector.memset(warm, 0.0)
        wps = psbig.tile([128, 512], f32, tag="big", name="warm_ps")
        NWARM = 6
        for w in range(NWARM):
            nc.tensor.matmul(wps, warm[:, 0:128], warm[:, 128:640],
                             start=True, stop=True)

        # ---- phase A: K_aug = [xj|1]^T [xj|1] -> K and sxj ------------
        K_ps = [psacc.tile([128, XJF], f32, tag="acc", name=f"K_ps{m}")
                for m in range(NCH)]
        for i in range(NT):
            for m in range(NCH):
                msl = slice(m * 128, (m + 1) * 128)
                nc.tensor.matmul(
                    K_ps[m][:, 0:C + 2], xjt[:, i, msl], xjt[:, i, 0:C + 2],
                    start=(i == 0), stop=(i == NT - 1),
                )
        K_sb = []
        sxjc = rowsp.tile([128, NCH], f16, tag="sxjc")
        for m in range(NCH):
            t = workp.tile([128, C], f16, tag=f"K{m}")
            nc.scalar.activation(
                out=t, in_=K_ps[m][:, 0:C],
                func=mybir.ActivationFunctionType.Identity, scale=1.0 / N)
            K_sb.append(t)
            nc.vector.tensor_copy(sxjc[:, m:m + 1], K_ps[m][:, C:C + 1])

        # ---- phase B: runtime rows a1, b2, scalar c2 ------------------
        rows_ps = psrow.tile([1, 2 * C + 8], f32, tag="rows")
        for k in range(NCH):
            nc.tensor.matmul(rows_ps[:, 0:C], sxjc[:, k:k + 1], lt(k),
                             start=(k == 0), stop=(k == NCH - 1))
        for k in range(NCH):
            nc.tensor.matmul(rows_ps[:, C:2 * C], sxjc[:, k:k + 1], rcw(k),
                             start=(k == 0), stop=(k == NCH - 1))
        for k in range(NCH):
            nc.tensor.matmul(rows_ps[:, 2 * C:2 * C + 1], sxjc[:, k:k + 1],
                             ptbc(k),
                             start=(k == 0), stop=(k == NCH - 1))
        a1row = rowsp.tile([1, C], f16, tag="a1row")
        nc.vector.scalar_tensor_tensor(
            out=a1row, in0=rows_ps[:, 0:C], scalar=1.0 / N,
            in1=aux[:, C:2 * C],
            op0=mybir.AluOpType.mult, op1=mybir.AluOpType.add)
        b2row = rowsp.tile([1, C], f16, tag="b2row")
        nc.vector.tensor_scalar_mul(b2row, rows_ps[:, C:2 * C], 1.0 / N)
        c2cell = rowsp.tile([1, 1], f16, tag="c2cell")
        nc.vector.tensor_scalar_mul(c2cell, rows_ps[:, 2 * C:2 * C + 1], 1.0 / N)

        # ---- phase C: T1 = (K/N) L'^T;  ET = R^T T1 + rank1 + I -------
        T1_sb = []
        for cb in range(NCH):
            t1_ps = psacc.tile([128, C], f32, tag="acc")
            csl = slice(cb * 128, (cb + 1) * 128)
            for jb in range(NCH):
                nc.tensor.matmul(t1_ps, K_sb[jb][:, csl], lt(jb),
                                 start=(jb == 0), stop=(jb == NCH - 1))
            t = workp.tile([128, C], f16, tag=f"T1{cb}")
            nc.scalar.copy(t, t1_ps)
            T1_sb.append(t)
        ET_sb = []
        for ob in range(NCH):
            et_ps = psacc.tile([128, C], f32, tag="acc")
            osl = slice(ob * 128, (ob + 1) * 128)
            for cb in range(NCH):
                nc.tensor.matmul(et_ps, rcw(cb, osl), T1_sb[cb],
                                 start=(cb == 0), stop=False)
            nc.tensor.matmul(et_ps, aux[:, osl], a1row, start=False, stop=False)
            nc.tensor.matmul(et_ps, b2row[:, osl], aux[:, C:2 * C],
                             start=False, stop=True)
            t = workp.tile([128, C], f16, tag=f"ET{ob}")
            nc.vector.tensor_add(t, et_ps, wm[:, ob, WM_ID])
            ET_sb.append(t)

        # ---- phase D: d = L'(K/N)ptb + c1 a1 + c2n wgb + W_b ----------
        col_ps = pscol.tile([128, 8], f32, tag="cols")
        for cb in range(NCH):
            csl = slice(cb * 128, (cb + 1) * 128)
            for jb in range(NCH):
                nc.tensor.matmul(col_ps[:, cb:cb + 1], K_sb[jb][:, csl],
                                 ptbc(jb),
                                 start=(jb == 0), stop=(jb == NCH - 1))
        kpc = rowsp.tile([128, NCH], f16, tag="kpc")
        nc.scalar.copy(kpc, col_ps[:, 0:NCH])
        for ob in range(NCH):
            osl = slice(ob * 128, (ob + 1) * 128)
            for cb in range(NCH):
                nc.tensor.matmul(col_ps[:, 2 + ob:3 + ob], lt(cb, osl),
                                 kpc[:, cb:cb + 1],
                                 start=(cb == 0), stop=False)
            nc.tensor.matmul(col_ps[:, 2 + ob:3 + ob], a1row[:, osl],
                             aux[:, 2 * C:2 * C + 1], start=False, stop=False)
            nc.tensor.matmul(col_ps[:, 2 + ob:3 + ob],
                             aux[:, C + ob * 128:C + (ob + 1) * 128],
                             c2cell, start=False, stop=True)
        dcol = rowsp.tile([128, NCH], f32, tag="dcol")
        nc.vector.tensor_add(dcol, col_ps[:, 2:2 + NCH], sm[:, :, 2])

        # ---- phase E: z0 = A xi; ACT adds d into z_t; DVE bn_stats ----
        # ---- reads the same PSUM tile in parallel ---------------------
        z_t = datap.tile([128, NCH, N], f32, tag="z")
        stats = [workp.tile([128, NZ, 6], f32, tag="bnst", name=f"stats{j}")
                 for j in range(NCH)]
        for tix in range(NZ):
            tsl = slice(tix * 512, (tix + 1) * 512)
            for j in range(NCH):
                jsl = slice(j * 128, (j + 1) * 128)
                z_ps = psbig.tile([128, 512], f32, tag="big")
                for k in range(NCH):
                    nc.tensor.matmul(z_ps, ET_sb[k][:, jsl], xi_t[:, k, tsl],
                                     start=(k == 0), stop=(k == NCH - 1))
                nc.scalar.activation(
                    out=z_t[:, j, tsl], in_=z_ps,
                    func=mybir.ActivationFunctionType.Identity,
                    bias=dcol[:, j:j + 1], scale=1.0)
                nc.vector.bn_stats(out=stats[j][:, tix, :], in_=z_ps)

        # stats of z = z0 + d: mean += d, var unchanged.
        # spack = (mean/8, (var + mean^2)/8) per chunk -> [128, 4]
        spack = rowsp.tile([128, 4], f32, tag="spack")
        mcols = rowsp.tile([128, NCH], f32, tag="mcols")
        for j in range(NCH):
            mv = rowsp.tile([128, 2], f32, tag="mv")
            nc.vector.bn_aggr(out=mv, in_=stats[j])
            nc.vector.tensor_add(mcols[:, j:j + 1], mv[:, 0:1], dcol[:, j:j + 1])
            nc.vector.tensor_scalar_mul(
                spack[:, 2 * j:2 * j + 1], mcols[:, j:j + 1], 1.0 / NCORES)
            nc.vector.scalar_tensor_tensor(
                out=spack[:, 2 * j + 1:2 * j + 2], in0=mcols[:, j:j + 1],
                scalar=mcols[:, j:j + 1], in1=mv[:, 1:2],
                op0=mybir.AluOpType.mult, op1=mybir.AluOpType.add)
            nc.vector.tensor_scalar_mul(
                spack[:, 2 * j + 1:2 * j + 2],
                spack[:, 2 * j + 1:2 * j + 2], 1.0 / NCORES)

        # ---- ONE ReduceScatter: input = own stats tiled 8x ------------
        cc_in = dramp.tile([NCORES * 128, 4], f32, tag="cc_in", name="cc_in")
        cc_out = dramp.tile([128, 4], f32, tag="cc_out", name="cc_out")
        nc.sync.dma_start(
            out=cc_in.rearrange("(r p) f -> p r f", p=128),
            in_=spack.unsqueeze(1).broadcast_to([128, NCORES, 4]))
        if skip_cc:
            nc.sync.dma_start(out=cc_out, in_=cc_in[0:128, :])
        else:
            nc.gpsimd.collective_compute(
                "ReduceScatter",
                mybir.AluOpType.add,
                replica_groups=[list(range(NCORES))],
                ins=[cc_in.opt()],
                outs=[cc_out.opt()],
            )
        ssum = rowsp.tile([128, 4], f32, tag="ssum")
        nc.sync.dma_start(out=ssum, in_=cc_out)

        # ---- normalize + affine + store (f16), quarters DVE || ACT ----
        for j in range(NCH):
            mcol = ssum[:, 2 * j:2 * j + 1]
            qcol = ssum[:, 2 * j + 1:2 * j + 2]
            nvcol = rowsp.tile([128, 1], f32, tag="nvcol")
            nc.vector.scalar_tensor_tensor(
                out=nvcol, in0=mcol, scalar=mcol, in1=qcol,
                op0=mybir.AluOpType.mult, op1=mybir.AluOpType.subtract)
            scol = rowsp.tile([128, 1], f32, tag="scol")
            nc.scalar.activation(
                out=scol, in_=nvcol, func=mybir.ActivationFunctionType.Sqrt,
                bias=eps, scale=-1.0)
            nc.vector.reciprocal(out=scol, in_=scol)
            acol = rowsp.tile([128, 1], f32, tag="acol")
            nc.vector.tensor_mul(acol, scol, sm[:, j, 0:1])
            bcol = rowsp.tile([128, 1], f32, tag="bcol")
            nc.vector.scalar_tensor_tensor(
                out=bcol, in0=mcol, scalar=acol, in1=sm[:, j, 1:2],
                op0=mybir.AluOpType.mult, op1=mybir.AluOpType.subtract)
            nbcol = rowsp.tile([128, 1], f32, tag="nbcol")
            nc.vector.tensor_scalar_mul(nbcol, bcol, -1.0)
            for q in range(4):
                qsl = slice(q * (N // 4), (q + 1) * (N // 4))
                o16 = outp.tile([128, N // 4], f16, tag="o16")
                if q % 2 == 0:
                    nc.vector.tensor_scalar(
                        out=o16, in0=z_t[:, j, qsl],
                        scalar1=acol, scalar2=bcol,
                        op0=mybir.AluOpType.mult, op1=mybir.AluOpType.subtract)
                else:
                    nc.scalar.activation(
                        out=o16, in_=z_t[:, j, qsl],
                        func=mybir.ActivationFunctionType.Identity,
                        bias=nbcol, scale=acol)
                nc.sync.dma_start(
                    out=out_d[j * 128:(j + 1) * 128, qsl], in_=o16)


_NC_CACHE: dict = {}


def _get_nc():
    if "nc" not in _NC_CACHE:
        nc = bacc.Bacc(
            "TRN2",
            target_bir_lowering=False,
            debug=False,
            enable_asserts=True,
            num_devices=NCORES,
        )
        build_kernel(nc)
        nc.compile()
        _NC_CACHE["nc"] = nc
    return _NC_CACHE["nc"]


def _make_in_maps(inputs: dict) -> list[dict]:
    xi = np.asarray(inputs["xi"], np.float32).reshape(B, C, N)
    xj = np.asarray(inputs["xj"], np.float32).reshape(B, C, N)
    g_w = np.asarray(inputs["g_w"], np.float32)
    g_b = np.asarray(inputs["g_b"], np.float32)
    t_w = np.asarray(inputs["theta_w"], np.float32)
    t_b = np.asarray(inputs["theta_b"], np.float32)
    p_w = np.asarray(inputs["phi_w"], np.float32)
    p_b = np.asarray(inputs["phi_b"], np.float32)
    W_w = np.asarray(inputs["W_w"], np.float32)
    W_b = np.asarray(inputs["W_b"], np.float32)
    gam = np.asarray(inputs["bn_gamma"], np.float32)
    bet = np.asarray(inputs["bn_beta"], np.float32)

    def chunked(a):  # [256, F] -> [128, 2, F]
        return np.ascontiguousarray(a.reshape(2, 128, -1).transpose(1, 0, 2))

    # host-folded weight products (constant folding, fp32)
    Lp = W_w @ g_w                      # L' = W G   (device uses K/N)
    R = p_w.T @ t_w                     # R = P^T T
    wgb = W_w @ g_b
    b1 = t_w.T @ p_b
    ptb = p_w.T @ t_b
    c1 = float(p_b @ t_b)

    wm = np.zeros((128, NCH, WM_F), np.float16)
    wm[:, :, 0:C] = chunked(Lp.T)
    wm[:, :, C:2 * C] = chunked(R)
    wm[:, :, 2 * C:3 * C] = chunked(np.eye(C, dtype=np.float32))
    wm[:, :, 3 * C] = ptb.reshape(2, 128).T
    aux = np.zeros((1, 2 * C + 8), np.float16)
    aux[0, 0:C] = b1.astype(np.float16)
    aux[0, C:2 * C] = wgb.astype(np.float16)
    aux[0, 2 * C] = np.float16(c1)
    sm = np.zeros((128, NCH, 3), np.float32)
    sm[:, :, 0] = gam.reshape(2, 128).T
    sm[:, :, 1] = bet.reshape(2, 128).T
    sm[:, :, 2] = W_b.reshape(2, 128).T

    in_maps = []
    for b in range(B):
        # layout-only transforms of the per-batch data (f16)
        xjta = np.zeros((128, NT, XJF), np.float16)
        xjta[:, :, 0:C] = xj[b].T.reshape(NT, 128, C).transpose(1, 0, 2)
        xjta[:, :, C] = 1.0
        xib = chunked(xi[b]).astype(np.float16)      # [128,2,4096]
        in_maps.append({
            "xjt": xjta, "xi": xib, "wm": wm, "aux": aux, "sm": sm,
        })
    return in_maps


def kernel(**inputs) -> np.ndarray:
    nc = _get_nc()
    in_maps = _make_in_maps(inputs)
    last_err = None
    for attempt in range(3):
        try:
            res = bass_utils.run_bass_kernel_spmd(
                nc, in_maps, core_ids=list(range(NCORES)),
            )
            break
        except Exception as e:  # transient device wedge: back off and retry
            last_err = e
            import time as _time
            _time.sleep(4.0 * (attempt + 1))
            try:
                import jax
                import jax.extend.backend as _jeb
                jax.clear_caches()
                _jeb.clear_backends()
            except Exception:
                pass
    else:
        raise last_err
    out = np.stack([res.results[c]["out"] for c in range(NCORES)])
    return np.ascontiguousarray(out.reshape(B, C, 64, 64).astype(np.float32))


if __name__ == "__main__":
    rng = np.random.default_rng(0)
    fake = {
        "xi": rng.standard_normal((B, C, 64, 64)).astype(np.float32),
        "xj": rng.standard_normal((B, C, 64, 64)).astype(np.float32),
        "g_w": (rng.standard_normal((C, C)) / 16).astype(np.float32),
        "g_b": (rng.standard_normal((C,)) / 16).astype(np.float32),
        "theta_w": (rng.standard_normal((C, C)) / 16).astype(np.float32),
        "theta_b": (rng.standard_normal((C,)) / 16).astype(np.float32),
        "phi_w": (rng.standard_normal((C, C)) / 16).astype(np.float32),
        "phi_b": (rng.standard_normal((C,)) / 16).astype(np.float32),
        "W_w": (rng.standard_normal((C, C)) / 16).astype(np.float32),
        "W_b": (rng.standard_normal((C,)) / 16).astype(np.float32),
        "bn_gamma": np.ones((C,), np.float32),
        "bn_beta": np.zeros((C,), np.float32),
    }
    out = kernel(**fake)
    print("out", out.shape, out.dtype, float(np.abs(out).mean()))
